# revision 17
# baseline (speedup 1.0000x reference)
"""NCC (local normalized cross-correlation) loss kernel for Trainium2.

8 NeuronCores, SPMD: D axis sharded (20 output slices/core + 4-slice halo,
zero-padded by host). Per core: partitions = H-chunk (h[0:128) and h[116:192),
overlapped so the H box-conv never crosses chunks), free = (d, w).

Pipeline: gpsimd DMA loads cast f32->bf16; I*I, J*J on ScalarE; I*J on
VectorE; D box-conv = box3(box3 stride 3) shifted adds on VectorE (bf16 2x
mode) into dedicated b9 tiles; W box-conv per field either as 9 w-shifted
accumulating matmuls on TensorE (fused with the H conv) or as ONE
sliding-window recurrence (tensor_tensor_scan) on VectorE -- split chosen to
balance engines (chunk0's K=128 matmuls are cheap, chunk1's K=76 are not);
H box-conv on TensorE as a banded-ones matmul; NCC post fused via
scalar_tensor_tensor reading PSUM directly, reciprocal on ScalarE,
per-partition sums via accum_out. Host sums 8 x [128,1] partials.
"""

import numpy as np

D, H, W = 160, 192, 224
WIN = 9
PAD = WIN // 2  # 4
NCORES = 8
DSH = D // NCORES  # 20
DIN = DSH + 2 * PAD  # 28
NSL = 2  # d slices per matmul chunk
NCOLS = NSL * W  # 448
NCHUNK = DSH // NSL  # 10
HCHUNKS = [(0, 128, 0, 120), (64, 128, 120, 72)]  # (h_in0, K, h_out0, M)
INV_N = 1.0 / float(WIN**3)
INV_SQRT_N = 1.0 / 27.0
SC_A = 729.0 / (19683.0 ** 0.5)  # 5.196...
SC_B = 1.0 / (19683.0 ** 0.5)
# Per chunk: which fields' W box-conv runs on TensorE (9 shifted matmuls);
# the rest use a VectorE scan. Field order: I, J, I2, J2, IJ.
PE_W = [(0, 1, 2, 3), (0, 1, 2, 3)]
# b9 tile layouts: PE-consumed tiles need 4+4 w-pads; scan inputs need 10+6.
WP_PE = 4 + W + 4  # 232
WP_SC = 10 + W + 6  # 240

_CACHE = {}


def _build():
    from contextlib import ExitStack

    import concourse.tile as tile
    from concourse import bacc, mybir

    f32 = mybir.dt.float32
    DT = mybir.dt.float16
    Alu = mybir.AluOpType
    Act = mybir.ActivationFunctionType

    nc = bacc.Bacc("TRN2", target_bir_lowering=False, debug=False)
    I_d = nc.dram_tensor("I_slab", [DIN, H, W], f32, kind="ExternalInput")
    J_d = nc.dram_tensor("J_slab", [DIN, H, W], f32, kind="ExternalInput")
    B_d = nc.dram_tensor("Bmat", [128, 192], DT, kind="ExternalInput")
    O_d = nc.dram_tensor("partials", [128, 1], f32, kind="ExternalOutput")

    def act_recip(out_ap, in_ap):
        eng = nc.scalar
        ins = [eng.lower_ap(in_ap)]
        for v in (0.0, 1.0, 0.0):  # bias, scale, alpha
            ins.append(mybir.ImmediateValue(dtype=f32, value=v))
        eng.add_instruction(
            mybir.InstActivation(
                name=nc.get_next_instruction_name(),
                func=Act.Reciprocal,
                ins=ins,
                outs=[eng.lower_ap(out_ap)],
            )
        )

    with tile.TileContext(nc) as tc, ExitStack() as ctx:
        const_p = ctx.enter_context(tc.tile_pool(name="const", bufs=1))
        field_p = ctx.enter_context(tc.tile_pool(name="field", bufs=1))
        s3_p = ctx.enter_context(tc.tile_pool(name="s3", bufs=2))
        b9_p = ctx.enter_context(tc.tile_pool(name="b9", bufs=1))
        tmp_p = ctx.enter_context(tc.tile_pool(name="tmp", bufs=1))
        psum_p = ctx.enter_context(tc.tile_pool(name="psum", bufs=6, space="PSUM"))

        Bt = const_p.tile([128, 192], DT)
        nc.sync.dma_start(Bt[:], B_d.ap())
        partials = const_p.tile([128, NCHUNK], f32)
        nc.vector.memset(partials[:], 0.0)

        for pci, (hin0, K, hout0, M) in enumerate(HCHUNKS):
            # ---- load with in-flight f32->bf16 cast ----
            fields = []
            for name, src in (("I", I_d), ("J", J_d)):
                ft = field_p.tile([128, DIN, W], DT, tag=f"f{name}")
                for q in range(4):
                    dq = np.s_[q * 7 : (q + 1) * 7]
                    src_ap = src.ap()[dq, hin0 : hin0 + K, :].rearrange(
                        "d h w -> h d w"
                    )
                    nc.gpsimd.dma_start(ft[:K, dq], src_ap)
                fields.append(ft)
            It, Jt = fields

            # ---- pointwise products ----
            I2 = field_p.tile([128, DIN, W], DT, tag="fI2")
            nc.scalar.activation(I2[:K], It[:K], Act.Square)
            J2 = field_p.tile([128, DIN, W], DT, tag="fJ2")
            nc.scalar.activation(J2[:K], Jt[:K], Act.Square)
            IJ = field_p.tile([128, DIN, W], DT, tag="fIJ")
            nc.vector.tensor_mul(IJ[:K], It[:K], Jt[:K])

            # ---- D box conv into b9 tiles; W box conv (scan fields) ----
            mm_rhs = []  # per field: (tile, wlp, is_scan)
            for fi, f in enumerate((It, Jt, I2, J2, IJ)):
                on_pe = fi in PE_W[pci]
                wp = WP_PE if on_pe else WP_SC
                wlp = 4 if on_pe else 10
                s3 = s3_p.tile([128, DIN - 2, W], DT, tag="s3")
                nc.vector.tensor_add(s3[:K], f[:K, 0 : DIN - 2], f[:K, 1 : DIN - 1])
                nc.vector.tensor_add(s3[:K], s3[:K], f[:K, 2:DIN])
                tag = f"b9_{fi}_c{pci}" if on_pe else f"b9s_{fi}"
                b9 = b9_p.tile([128, DSH, wp], DT, tag=tag)
                nc.gpsimd.memset(b9[:K, :, 0:wlp], 0.0)
                nc.gpsimd.memset(b9[:K, :, wlp + W : wp], 0.0)
                dat = np.s_[wlp : wlp + W]
                nc.vector.tensor_add(
                    b9[:K, :, dat], s3[:K, 0:DSH], s3[:K, 3 : 3 + DSH]
                )
                nc.vector.tensor_add(
                    b9[:K, :, dat], b9[:K, :, dat], s3[:K, 6 : 6 + DSH]
                )
                if on_pe:
                    mm_rhs.append((b9, wlp, False))
                else:
                    blen = DSH * wp
                    sf = b9_p.tile([128, DSH, wp], DT, tag=f"so_{fi}")
                    f_flat = b9[:K].rearrange("p a b -> p (a b)")
                    s_flat = sf[:K].rearrange("p a b -> p (a b)")
                    nc.vector.tensor_tensor_scan(
                        s_flat[:, 5 : blen - 4],
                        f_flat[:, 9:blen],
                        f_flat[:, 0 : blen - 9],
                        0.0,
                        Alu.add,
                        Alu.subtract,
                    )
                    mm_rhs.append((sf, wlp, True))

            # ---- H (+W for PE fields) box conv on PE; evacuate with folded
            # scalings into staging; batched NCC post every G chunks ----
            G = 2
            lhsT = Bt[:K, hout0 : hout0 + M]
            GCOLS = G * NCOLS
            for nch in range(NCHUNK):
                dsl = np.s_[nch * NSL : (nch + 1) * NSL]
                ps = []
                for fi, (ft, wlp, scanned) in enumerate(mm_rhs):
                    pt = psum_p.tile([M, NCOLS], f32, tag="ps")
                    if scanned:
                        nc.tensor.matmul(
                            pt[:],
                            lhsT,
                            ft[:K, dsl, wlp : wlp + W],
                            start=True,
                            stop=True,
                        )
                    else:
                        for k in range(WIN):
                            rhs = ft[:K, dsl, k : k + W]
                            nc.tensor.matmul(
                                pt[:], lhsT, rhs, start=(k == 0), stop=(k == WIN - 1)
                            )
                    ps.append(pt)
                a, b, c, d, e = ps  # I_sum, J_sum, I2_sum, J2_sum, IJ_sum

                # evacuate PSUM -> fp16 staging via ScalarE, scalings folded
                if nch % G == 0:
                    sta = tmp_p.tile([128, GCOLS], DT, tag="st_a")
                    stb = tmp_p.tile([128, GCOLS], DT, tag="st_b")
                    stc = tmp_p.tile([128, GCOLS], DT, tag="st_c")
                    std = tmp_p.tile([128, GCOLS], DT, tag="st_d")
                    ste = tmp_p.tile([128, GCOLS], DT, tag="st_e")
                g = nch % G
                gsl = np.s_[g * NCOLS : (g + 1) * NCOLS]
                nc.scalar.mul(sta[:M, gsl], a[:], INV_N)  # a/729
                nc.scalar.copy(stb[:M, gsl], b[:])
                nc.scalar.mul(stc[:M, gsl], c[:], INV_SQRT_N)  # c/27
                nc.scalar.mul(std[:M, gsl], d[:], INV_SQRT_N)  # d/27
                nc.scalar.copy(ste[:M, gsl], e[:])

                if g != G - 1:
                    continue
                # batched post on [M, GCOLS]; every VectorE op is 2x fp16 TT
                t1 = tmp_p.tile([128, GCOLS], DT, tag="t1")
                nc.vector.tensor_mul(t1[:], sta[:], stb[:])  # ab/729
                cross = tmp_p.tile([128, GCOLS], DT, tag="cross")
                nc.vector.tensor_sub(cross[:], ste[:], t1[:])
                t2 = tmp_p.tile([128, GCOLS], DT, tag="t2")
                # (a/729 * SC_A)^2 = a^2/19683 = (a^2/729)/27
                nc.scalar.activation(t2[:], sta[:], Act.Square, scale=SC_A)
                ivar = tmp_p.tile([128, GCOLS], DT, tag="ivar")
                nc.vector.tensor_sub(ivar[:], stc[:], t2[:])  # Ivar/27
                t3 = tmp_p.tile([128, GCOLS], DT, tag="t3")
                nc.scalar.activation(t3[:], stb[:], Act.Square, scale=SC_B)
                jvar = tmp_p.tile([128, GCOLS], DT, tag="jvar")
                nc.vector.tensor_sub(jvar[:], std[:], t3[:])  # Jvar/27
                den = tmp_p.tile([128, GCOLS], DT, tag="t3")
                nc.vector.tensor_mul(den[:], ivar[:], jvar[:])  # IvarJvar/729
                rec = tmp_p.tile([128, GCOLS], DT, tag="rec")
                act_recip(rec[:M], den[:M])  # 729/(IvarJvar)
                num = tmp_p.tile([128, GCOLS], DT, tag="t2")
                nc.scalar.activation(num[:], cross[:], Act.Square)
                scrap = tmp_p.tile([128, GCOLS], DT, tag="t1")
                bi = pci * (NCHUNK // G) + nch // G
                nc.vector.scalar_tensor_tensor(
                    scrap[:M],
                    num[:M],
                    INV_N,
                    rec[:M],
                    Alu.mult,
                    Alu.mult,
                    accum_out=partials[:M, bi : bi + 1],
                )

        out_sb = const_p.tile([128, 1], f32)
        nc.vector.reduce_sum(out_sb[:], partials[:], axis=mybir.AxisListType.X)
        nc.sync.dma_start(O_d.ap(), out_sb[:])

    nc.compile()
    return nc


def _prepare_inputs(I, J):
    import ml_dtypes

    I3 = np.asarray(I, dtype=np.float32).reshape(D, H, W)
    J3 = np.asarray(J, dtype=np.float32).reshape(D, H, W)

    Bmat = np.zeros((128, 192), dtype=np.float32)
    for k in range(128):
        for m in range(120):
            if abs(k - m) <= PAD:
                Bmat[k, m] = 1.0
    for k in range(128):
        for m in range(72):
            if abs((64 + k) - (120 + m)) <= PAD:
                Bmat[k, 120 + m] = 1.0
    Bmat = Bmat.astype(np.float16)

    in_maps = []
    for c in range(NCORES):
        lo, hi = c * DSH - PAD, c * DSH + DSH + PAD
        clo, chi = max(lo, 0), min(hi, D)
        islab = np.zeros((DIN, H, W), dtype=np.float32)
        jslab = np.zeros((DIN, H, W), dtype=np.float32)
        islab[clo - lo : clo - lo + chi - clo] = I3[clo:chi]
        jslab[clo - lo : clo - lo + chi - clo] = J3[clo:chi]
        in_maps.append({"I_slab": islab, "J_slab": jslab, "Bmat": Bmat})
    return in_maps


def kernel(I, J, _trace=False, _tmpdir=None):
    from concourse.bass_utils import run_bass_kernel_spmd

    if "nc" not in _CACHE:
        _CACHE["nc"] = _build()
    nc = _CACHE["nc"]
    in_maps = _prepare_inputs(I, J)
    kw = {}
    if _trace:
        kw = dict(trace=True, tmpdir=_tmpdir)
    res = run_bass_kernel_spmd(nc, in_maps, core_ids=list(range(NCORES)), **kw)
    total = np.float64(0.0)
    for c in range(NCORES):
        total += np.float64(res.results[c]["partials"].sum())
    out = np.float32(-(total / float(D * H * W)))
    if _trace:
        return np.asarray(out, dtype=np.float32), res
    return np.asarray(out, dtype=np.float32)


# revision 19
# speedup vs baseline: 1.0174x; 1.0174x over previous
"""NCC (local normalized cross-correlation) loss kernel for Trainium2.

8 NeuronCores, SPMD: D axis sharded (20 output slices/core + 4-slice halo,
zero-padded by host). Per core: partitions = H-chunk (h[0:128) and h[116:192),
overlapped so the H box-conv never crosses chunks), free = (d, w).

Pipeline: gpsimd DMA loads cast f32->bf16; I*I, J*J on ScalarE; I*J on
VectorE; D box-conv = box3(box3 stride 3) shifted adds on VectorE (bf16 2x
mode) into dedicated b9 tiles; W box-conv per field either as 9 w-shifted
accumulating matmuls on TensorE (fused with the H conv) or as ONE
sliding-window recurrence (tensor_tensor_scan) on VectorE -- split chosen to
balance engines (chunk0's K=128 matmuls are cheap, chunk1's K=76 are not);
H box-conv on TensorE as a banded-ones matmul; NCC post fused via
scalar_tensor_tensor reading PSUM directly, reciprocal on ScalarE,
per-partition sums via accum_out. Host sums 8 x [128,1] partials.
"""

import numpy as np

D, H, W = 160, 192, 224
WIN = 9
PAD = WIN // 2  # 4
NCORES = 8
DSH = D // NCORES  # 20
DIN = DSH + 2 * PAD  # 28
NSL = 2  # d slices per matmul chunk
NCOLS = NSL * W  # 448
NCHUNK = DSH // NSL  # 10
HCHUNKS = [(0, 128, 0, 120), (64, 128, 120, 72)]  # (h_in0, K, h_out0, M)
INV_N = 1.0 / float(WIN**3)
INV_SQRT_N = 1.0 / 27.0
SC_A = 729.0 / (19683.0 ** 0.5)  # 5.196...
SC_B = 1.0 / (19683.0 ** 0.5)
# Per chunk: which fields' W box-conv runs on TensorE (9 shifted matmuls);
# the rest use a VectorE scan. Field order: I, J, I2, J2, IJ.
PE_W = [(0, 1, 2, 3), (0, 1, 2)]
# b9 tile layouts: PE-consumed tiles need 4+4 w-pads; scan inputs need 10+6.
WP_PE = 4 + W + 4  # 232
WP_SC = 10 + W + 6  # 240

_CACHE = {}


def _build():
    from contextlib import ExitStack

    import concourse.tile as tile
    from concourse import bacc, mybir

    f32 = mybir.dt.float32
    DT = mybir.dt.float16
    Alu = mybir.AluOpType
    Act = mybir.ActivationFunctionType

    nc = bacc.Bacc("TRN2", target_bir_lowering=False, debug=False)
    I_d = nc.dram_tensor("I_slab", [DIN, H, W], f32, kind="ExternalInput")
    J_d = nc.dram_tensor("J_slab", [DIN, H, W], f32, kind="ExternalInput")
    B_d = nc.dram_tensor("Bmat", [128, 192], DT, kind="ExternalInput")
    O_d = nc.dram_tensor("partials", [128, 1], f32, kind="ExternalOutput")

    def act_recip(out_ap, in_ap):
        eng = nc.scalar
        ins = [eng.lower_ap(in_ap)]
        for v in (0.0, 1.0, 0.0):  # bias, scale, alpha
            ins.append(mybir.ImmediateValue(dtype=f32, value=v))
        eng.add_instruction(
            mybir.InstActivation(
                name=nc.get_next_instruction_name(),
                func=Act.Reciprocal,
                ins=ins,
                outs=[eng.lower_ap(out_ap)],
            )
        )

    with tile.TileContext(nc) as tc, ExitStack() as ctx:
        const_p = ctx.enter_context(tc.tile_pool(name="const", bufs=1))
        field_p = ctx.enter_context(tc.tile_pool(name="field", bufs=1))
        s3_p = ctx.enter_context(tc.tile_pool(name="s3", bufs=2))
        b9_p = ctx.enter_context(tc.tile_pool(name="b9", bufs=1))
        tmp_p = ctx.enter_context(tc.tile_pool(name="tmp", bufs=1))
        psum_p = ctx.enter_context(tc.tile_pool(name="psum", bufs=6, space="PSUM"))

        Bt = const_p.tile([128, 192], DT)
        nc.sync.dma_start(Bt[:], B_d.ap())
        partials = const_p.tile([128, NCHUNK], f32)
        nc.vector.memset(partials[:], 0.0)

        for pci, (hin0, K, hout0, M) in enumerate(HCHUNKS):
            # ---- load with in-flight f32->bf16 cast ----
            fields = []
            for name, src in (("I", I_d), ("J", J_d)):
                ft = field_p.tile([128, DIN, W], DT, tag=f"f{name}")
                for q in range(4):
                    dq = np.s_[q * 7 : (q + 1) * 7]
                    src_ap = src.ap()[dq, hin0 : hin0 + K, :].rearrange(
                        "d h w -> h d w"
                    )
                    nc.gpsimd.dma_start(ft[:K, dq], src_ap)
                fields.append(ft)
            It, Jt = fields

            # ---- pointwise products ----
            I2 = field_p.tile([128, DIN, W], DT, tag="fI2")
            nc.scalar.activation(I2[:K], It[:K], Act.Square)
            J2 = field_p.tile([128, DIN, W], DT, tag="fJ2")
            nc.scalar.activation(J2[:K], Jt[:K], Act.Square)
            IJ = field_p.tile([128, DIN, W], DT, tag="fIJ")
            nc.vector.tensor_mul(IJ[:K], It[:K], Jt[:K])

            # ---- D box conv into b9 tiles; W box conv (scan fields) ----
            mm_rhs = []  # per field: (tile, wlp, is_scan)
            for fi, f in enumerate((It, Jt, I2, J2, IJ)):
                on_pe = fi in PE_W[pci]
                wp = WP_PE if on_pe else WP_SC
                wlp = 4 if on_pe else 10
                s3 = s3_p.tile([128, DIN - 2, W], DT, tag="s3")
                nc.vector.tensor_add(s3[:K], f[:K, 0 : DIN - 2], f[:K, 1 : DIN - 1])
                nc.vector.tensor_add(s3[:K], s3[:K], f[:K, 2:DIN])
                tag = f"b9_{fi}_c{pci}" if on_pe else f"b9s_{fi}"
                b9 = b9_p.tile([128, DSH, wp], DT, tag=tag)
                nc.gpsimd.memset(b9[:K, :, 0:wlp], 0.0)
                nc.gpsimd.memset(b9[:K, :, wlp + W : wp], 0.0)
                dat = np.s_[wlp : wlp + W]
                nc.vector.tensor_add(
                    b9[:K, :, dat], s3[:K, 0:DSH], s3[:K, 3 : 3 + DSH]
                )
                nc.vector.tensor_add(
                    b9[:K, :, dat], b9[:K, :, dat], s3[:K, 6 : 6 + DSH]
                )
                if on_pe:
                    mm_rhs.append((b9, wlp, False))
                else:
                    blen = DSH * wp
                    sf = b9_p.tile([128, DSH, wp], DT, tag=f"so_{fi}")
                    f_flat = b9[:K].rearrange("p a b -> p (a b)")
                    s_flat = sf[:K].rearrange("p a b -> p (a b)")
                    nc.vector.tensor_tensor_scan(
                        s_flat[:, 5 : blen - 4],
                        f_flat[:, 9:blen],
                        f_flat[:, 0 : blen - 9],
                        0.0,
                        Alu.add,
                        Alu.subtract,
                    )
                    mm_rhs.append((sf, wlp, True))

            # ---- H (+W for PE fields) box conv on PE; evacuate with folded
            # scalings into staging; batched NCC post every G chunks ----
            G = 2
            lhsT = Bt[:K, hout0 : hout0 + M]
            GCOLS = G * NCOLS
            for nch in range(NCHUNK):
                dsl = np.s_[nch * NSL : (nch + 1) * NSL]
                ps = []
                for fi, (ft, wlp, scanned) in enumerate(mm_rhs):
                    pt = psum_p.tile([M, NCOLS], f32, tag="ps")
                    if scanned:
                        nc.tensor.matmul(
                            pt[:],
                            lhsT,
                            ft[:K, dsl, wlp : wlp + W],
                            start=True,
                            stop=True,
                        )
                    else:
                        for k in range(WIN):
                            rhs = ft[:K, dsl, k : k + W]
                            nc.tensor.matmul(
                                pt[:], lhsT, rhs, start=(k == 0), stop=(k == WIN - 1)
                            )
                    ps.append(pt)
                a, b, c, d, e = ps  # I_sum, J_sum, I2_sum, J2_sum, IJ_sum

                # evacuate PSUM -> fp16 staging via ScalarE, scalings folded
                if nch % G == 0:
                    sta = tmp_p.tile([128, GCOLS], DT, tag="st_a")
                    stb = tmp_p.tile([128, GCOLS], DT, tag="st_b")
                    stc = tmp_p.tile([128, GCOLS], DT, tag="st_c")
                    std = tmp_p.tile([128, GCOLS], DT, tag="st_d")
                    ste = tmp_p.tile([128, GCOLS], DT, tag="st_e")
                g = nch % G
                gsl = np.s_[g * NCOLS : (g + 1) * NCOLS]
                nc.scalar.mul(sta[:M, gsl], a[:], INV_N)  # a/729
                nc.scalar.copy(stb[:M, gsl], b[:])
                nc.scalar.mul(stc[:M, gsl], c[:], INV_SQRT_N)  # c/27
                nc.scalar.mul(std[:M, gsl], d[:], INV_SQRT_N)  # d/27
                nc.scalar.copy(ste[:M, gsl], e[:])

                if g != G - 1:
                    continue
                # batched post on [M, GCOLS]; every VectorE op is 2x fp16 TT
                t1 = tmp_p.tile([128, GCOLS], DT, tag="t1")
                nc.vector.tensor_mul(t1[:], sta[:], stb[:])  # ab/729
                cross = tmp_p.tile([128, GCOLS], DT, tag="cross")
                nc.vector.tensor_sub(cross[:], ste[:], t1[:])
                t2 = tmp_p.tile([128, GCOLS], DT, tag="t2")
                # (a/729 * SC_A)^2 = a^2/19683 = (a^2/729)/27
                nc.scalar.activation(t2[:], sta[:], Act.Square, scale=SC_A)
                ivar = tmp_p.tile([128, GCOLS], DT, tag="ivar")
                nc.vector.tensor_sub(ivar[:], stc[:], t2[:])  # Ivar/27
                t3 = tmp_p.tile([128, GCOLS], DT, tag="t3")
                nc.scalar.activation(t3[:], stb[:], Act.Square, scale=SC_B)
                jvar = tmp_p.tile([128, GCOLS], DT, tag="jvar")
                nc.vector.tensor_sub(jvar[:], std[:], t3[:])  # Jvar/27
                den = tmp_p.tile([128, GCOLS], DT, tag="t3")
                nc.vector.tensor_mul(den[:], ivar[:], jvar[:])  # IvarJvar/729
                rec = tmp_p.tile([128, GCOLS], DT, tag="rec")
                act_recip(rec[:M], den[:M])  # 729/(IvarJvar)
                num = tmp_p.tile([128, GCOLS], DT, tag="t2")
                nc.scalar.activation(num[:], cross[:], Act.Square)
                scrap = tmp_p.tile([128, GCOLS], DT, tag="t1")
                bi = pci * (NCHUNK // G) + nch // G
                nc.vector.scalar_tensor_tensor(
                    scrap[:M],
                    num[:M],
                    INV_N,
                    rec[:M],
                    Alu.mult,
                    Alu.mult,
                    accum_out=partials[:M, bi : bi + 1],
                )

        out_sb = const_p.tile([128, 1], f32)
        nc.vector.reduce_sum(out_sb[:], partials[:], axis=mybir.AxisListType.X)
        nc.sync.dma_start(O_d.ap(), out_sb[:])

    nc.compile()
    return nc


def _prepare_inputs(I, J):
    import ml_dtypes

    I3 = np.asarray(I, dtype=np.float32).reshape(D, H, W)
    J3 = np.asarray(J, dtype=np.float32).reshape(D, H, W)

    Bmat = np.zeros((128, 192), dtype=np.float32)
    for k in range(128):
        for m in range(120):
            if abs(k - m) <= PAD:
                Bmat[k, m] = 1.0
    for k in range(128):
        for m in range(72):
            if abs((64 + k) - (120 + m)) <= PAD:
                Bmat[k, 120 + m] = 1.0
    Bmat = Bmat.astype(np.float16)

    in_maps = []
    for c in range(NCORES):
        lo, hi = c * DSH - PAD, c * DSH + DSH + PAD
        clo, chi = max(lo, 0), min(hi, D)
        islab = np.zeros((DIN, H, W), dtype=np.float32)
        jslab = np.zeros((DIN, H, W), dtype=np.float32)
        islab[clo - lo : clo - lo + chi - clo] = I3[clo:chi]
        jslab[clo - lo : clo - lo + chi - clo] = J3[clo:chi]
        in_maps.append({"I_slab": islab, "J_slab": jslab, "Bmat": Bmat})
    return in_maps


def kernel(I, J, _trace=False, _tmpdir=None):
    from concourse.bass_utils import run_bass_kernel_spmd

    if "nc" not in _CACHE:
        _CACHE["nc"] = _build()
    nc = _CACHE["nc"]
    in_maps = _prepare_inputs(I, J)
    kw = {}
    if _trace:
        kw = dict(trace=True, tmpdir=_tmpdir)
    res = run_bass_kernel_spmd(nc, in_maps, core_ids=list(range(NCORES)), **kw)
    total = np.float64(0.0)
    for c in range(NCORES):
        total += np.float64(res.results[c]["partials"].sum())
    out = np.float32(-(total / float(D * H * W)))
    if _trace:
        return np.asarray(out, dtype=np.float32), res
    return np.asarray(out, dtype=np.float32)
